# revision 1
# baseline (speedup 1.0000x reference)
"""Trainium2 Bass kernel for nn_AlignModel.

Computes out[b, j, i] = sigmoid(simp[b,j]·w_s + orig[b,i]·w_o + bias) where
orig/simp are the two halves of prop_state[b] ([B, 2S, D] -> [B,S,D] each),
w_o = W[0,:D], w_s = W[0,D:].

Sharding: data-parallel over batch B=8 across the 8 NeuronCores. Each core:
  in  x   [4096, 512] f32  (= prop_state[b])
  in  w   [1, 1024]   f32
  in  bvec[1, 1]      f32
  out out [2048, 2048] f32 (= sigmoid(s_s[:,None] + s_o[None,:] + b))

Structure (from ~20 NTFF-profile iterations; best measured 87.6 us):
  - Concurrent DMAs in a queue drain round-robin, so all transfers finish
    together at ~total/bandwidth.  The orig chunks therefore use GEOMETRIC
    sizes (1,1,2,4,4,4 tiles): early chunks surface quickly so the DVE
    multiply pipeline starts ~7 us sooner, while the aggregate stream
    still runs at full rate.  simp loads queue behind on the same Sync
    FIFO, WAW-gated (tiny DVE writes into their tiles) so they start only
    mid-phase-1a; output stores follow, keeping the DMA pipe continuously
    busy from first load to last store.
  - The orig half is consumed partition-outer (i = p*16 + n): contiguous
    per-partition input descriptors, and s_o[128,16] scatters straight
    into the broadcast row [1,2048] with tiny strided DMAs (no transpose),
    then one ScalarE cast to fp16.
  - so_row -> PSUM [128,2048] replication via rank-1 fp16 PE matmuls
    (fast even HAM-cold); the bias b is PSUM-seeded first and s_o
    accumulates on top.
  - Dot products: DVE tensor_mul + ScalarE Copy-with-accum (orig) / DVE
    tensor_reduce (simp); ScalarE is reserved for phase-2 sigmoids.
  - Each output row-tile is ONE ScalarE op
      out_t = Sigmoid(s_o_bcast + bias_col_t)   (PSUM -> SBUF).
    The first and last row-tiles ship as single 1 MiB stores (gated on one
    sigmoid, starting the drain earlier); the rest leave as 2 MiB pairs.
"""

import numpy as np

import concourse.mybir as mybir
from concourse import bacc, bass_utils
from concourse.tile import TileContext

P = 128          # partitions
D = 512          # feature dim
S = 2048         # sents
NT = S // P      # 16 tiles per half
OCHUNKS = [1, 1, 2, 4, 4, 4]   # orig tiles per chunk
SCH = 4          # simp tiles per chunk (1 MiB)
NSC = NT // SCH
NCORES = 8
F32 = mybir.dt.float32


def _kernel_body(tc, out, x, w, bvec):
    nc = tc.nc
    # orig half, partition-outer: i = p*NT + n
    xo_re = x[0:S, :].rearrange("(p n) d -> p n d", n=NT)
    # simp half, partition-inner: j = n*P + p  (bias needs column layout)
    xs_re = x[S:2 * S, :].rearrange("(n p) d -> p n d", p=P)

    with (
        tc.tile_pool(name="consts", bufs=1) as cpool,
        tc.tile_pool(name="xin", bufs=1) as xpool,
        tc.tile_pool(name="scratch", bufs=4) as spool,
        tc.tile_pool(name="outbuf", bufs=4) as opool,
        tc.tile_pool(name="psum", bufs=1, space="PSUM") as ppool,
    ):
        # --- orig input stream: geometric chunks, all in flight at once ---
        xo_tiles = []
        n0 = 0
        for c, sz in enumerate(OCHUNKS):
            xo = xpool.tile([P, sz, D], F32, tag=f"xo{c}", name=f"xo{c}")
            nc.sync.dma_start(out=xo, in_=xo_re[:, n0:n0 + sz, :])
            xo_tiles.append(xo)
            n0 += sz

        # simp tiles; their loads go on the Sync queue behind the orig
        # chunks but are gated by a tiny DVE write into each tile (WAW dep)
        # so the transfers only start once phase 1a is nearly done -- an
        # ungated DMA would be scheduled at t=0 and starve the orig stream.
        xs_tiles = [
            xpool.tile([P, SCH, D], F32, tag=f"xs{g}", name=f"xs{g}")
            for g in range(NSC)
        ]

        # w / b replicated across partitions by zero-stride DMA (SWDGE);
        # w_o first since it gates the first multiply.
        w_bc = cpool.tile([P, 2 * D], F32, tag="wbc")
        nc.gpsimd.dma_start(out=w_bc[:, 0:D],
                            in_=w[:, 0:D].broadcast_to([P, D]))
        nc.gpsimd.dma_start(out=w_bc[:, D:2 * D],
                            in_=w[:, D:2 * D].broadcast_to([P, D]))
        ones_row = cpool.tile([1, P], mybir.dt.float16, tag="ones")
        nc.gpsimd.memset(ones_row, 1.0)

        s_o_mat = cpool.tile([P, NT], F32, tag="somat")   # s_o[p*16+n] @ [p,n]
        s_sb_mat = cpool.tile([P, NT], F32, tag="ssmat")  # s_s + b, col t
        so_rowf = cpool.tile([1, S], F32, tag="sorowf")   # f32 scatter dest
        so_row = cpool.tile([1, S], mybir.dt.float16, tag="sorow")
        b_sb = cpool.tile([1, 1], F32, tag="bsb")
        nc.sync.dma_start(out=b_sb, in_=bvec)
        b_row = cpool.tile([1, 512], mybir.dt.float16, tag="brow")
        nc.gpsimd.memset(b_row, 0.0)
        nc.vector.tensor_scalar_add(b_row, b_row, b_sb)
        sob_psum = ppool.tile([P, S], F32, tag="sob")     # s_o on every row

        # --- phase 1a: orig half -> s_o -> so_row ---
        n0 = 0
        for c, sz in enumerate(OCHUNKS):
            xo = xo_tiles[c]
            for blk in range(sz):
                t = n0 + blk
                prod = spool.tile([P, D], F32, tag="prod", name=f"po{t}")
                nc.vector.tensor_mul(out=prod, in0=xo[:, blk, :],
                                     in1=w_bc[:, 0:D])
                nc.scalar.activation(
                    prod, prod, mybir.ActivationFunctionType.Copy,
                    accum_out=s_o_mat[:, t:t + 1])
                if t in (8, 10, 12):
                    gs = {8: (0,), 10: (1,), 12: (2, 3)}[t]
                    for g in gs:
                        nc.vector.tensor_copy(
                            out=xs_tiles[g][0:1, 0, 0:1],
                            in_=prod[0:1, 0:1])
            src = s_o_mat[:, n0:n0 + sz]
            dst = so_rowf.rearrange("o (p n) -> o p n", n=NT)[:, :, n0:n0 + sz]
            nc.scalar.dma_start(out=dst, in_=src)
            n0 += sz

        # single f32 -> fp16 cast on ScalarE (engine-local, no DMA semaphore
        # on the critical chain)
        nc.scalar.copy(so_row, so_rowf)

        # simp loads: queued on Sync behind the orig chunks, released by the
        # gate writes above
        for g in range(NSC):
            nc.sync.dma_start(out=xs_tiles[g],
                              in_=xs_re[:, g * SCH:(g + 1) * SCH, :])

        # --- broadcast b + s_o across partitions via rank-1 matmuls: the b
        # seed runs early (start=True), s_o accumulates on top ---
        for j in range(S // 512):
            nc.tensor.matmul(sob_psum[:, j * 512:(j + 1) * 512], ones_row,
                             b_row, start=True, stop=False)
        for j in range(S // 512):
            nc.tensor.matmul(sob_psum[:, j * 512:(j + 1) * 512], ones_row,
                             so_row[:, j * 512:(j + 1) * 512],
                             start=False, stop=True)

        # --- phase 1b + 2: simp half -> s_s + b, then outputs ---
        o_sb = None
        for g in range(NSC):
            xs = xs_tiles[g]
            for blk in range(SCH):
                t = g * SCH + blk
                prod = spool.tile([P, D], F32, tag="prod", name=f"ps{t}")
                nc.vector.tensor_mul(out=prod, in0=xs[:, blk, :],
                                     in1=w_bc[:, D:2 * D])
                nc.vector.tensor_reduce(
                    s_sb_mat[:, t:t + 1], prod,
                    axis=mybir.AxisListType.X, op=mybir.AluOpType.add)
            for blk in range(SCH):
                t = g * SCH + blk
                # tiles 0 and 15 ship as single 1 MiB stores so the output
                # stream starts one sigmoid (~2us) earlier; the rest pair up
                if t in (0, NT - 1):
                    o_sb = opool.tile([P, 2, S], F32, tag="osb",
                                      name=f"osingle{t}")
                    nc.scalar.activation(
                        o_sb[:, 0, :], sob_psum,
                        mybir.ActivationFunctionType.Sigmoid,
                        bias=s_sb_mat[:, t:t + 1], scale=1.0)
                    nc.sync.dma_start(out=out[t * P:(t + 1) * P, :],
                                      in_=o_sb[:, 0, :])
                    continue
                q = (t - 1) % 2
                if q == 0:
                    o_sb = opool.tile([P, 2, S], F32, tag="osb",
                                      name=f"opair{t // 2}")
                nc.scalar.activation(
                    o_sb[:, q, :], sob_psum,
                    mybir.ActivationFunctionType.Sigmoid,
                    bias=s_sb_mat[:, t:t + 1],
                    scale=1.0,
                )
                if q == 1:
                    r0 = (t - 1) * P
                    dst = out[r0:r0 + 2 * P, :].rearrange(
                        "(q p) i -> p q i", p=P)
                    nc.sync.dma_start(out=dst, in_=o_sb)


def build_program():
    nc = bacc.Bacc(
        "TRN2",
        debug=False,
        target_bir_lowering=False,
        num_devices=NCORES,
    )
    x = nc.dram_tensor("x", [2 * S, D], F32, kind="ExternalInput").ap()
    w = nc.dram_tensor("w", [1, 2 * D], F32, kind="ExternalInput").ap()
    bvec = nc.dram_tensor("bvec", [1, 1], F32, kind="ExternalInput").ap()
    out = nc.dram_tensor("out", [S, S], F32, kind="ExternalOutput").ap()
    with TileContext(nc) as tc:
        _kernel_body(tc, out, x, w, bvec)
    nc.compile()
    return nc


_PROGRAM = None


def _get_program():
    global _PROGRAM
    if _PROGRAM is None:
        _PROGRAM = build_program()
    return _PROGRAM


def make_in_maps(prop_state, W, b):
    prop = np.ascontiguousarray(np.asarray(prop_state, dtype=np.float32))
    w = np.ascontiguousarray(np.asarray(W, dtype=np.float32).reshape(1, 2 * D))
    bv = np.ascontiguousarray(np.asarray(b, dtype=np.float32).reshape(1, 1))
    assert prop.shape == (NCORES, 2 * S, D), prop.shape
    return [{"x": prop[i], "w": w, "bvec": bv} for i in range(NCORES)]


def kernel(A, prop_state, W, b, _trace=False):
    nc = _get_program()
    in_maps = make_in_maps(prop_state, W, b)
    res = bass_utils.run_bass_kernel_spmd(
        nc, in_maps, core_ids=list(range(NCORES)), trace=_trace)
    out = np.stack([res.results[i]["out"] for i in range(NCORES)], axis=0)
    if _trace:
        kernel.last_results = res
    return out



# revision 2
# speedup vs baseline: 1.0990x; 1.0990x over previous
"""Trainium2 Bass kernel for nn_AlignModel.

Computes out[b, j, i] = sigmoid(simp[b,j]·w_s + orig[b,i]·w_o + bias) where
orig/simp are the two halves of prop_state[b] ([B, 2S, D] -> [B,S,D] each),
w_o = W[0,:D], w_s = W[0,D:].

Sharding: data-parallel over batch B=8 across the 8 NeuronCores. Each core:
  in  x   [4096, 512] f16  (= prop_state[b], host-cast to fp16)
  in  w   [1, 1024]   f16
  in  bvec[1, 1]      f32
  out out [2048, 2048] f16 (= sigmoid(s_s[:,None] + s_o[None,:] + b)),
                           host-upcast to f32.

The 2e-2 rel-err gate admits half precision end to end: sigmoid outputs lie
in (0,1) where fp16 has ~5e-4 relative error, and the 512-length dots with
fp16 inputs / f32 accumulation carry ~1e-4 absolute score error.  Halving
both streams cuts per-core HBM traffic 24 MiB -> 12.6 MiB (~35 us at the
358 GB/s per-core HBM limit).

Engine split (vs the f32 baseline where ScalarE carried 49 us):
  - DVE: all 32 dot tiles (fp16 tensor_mul at 2x + tensor_reduce at 1x,
    ~1 us/tile) plus the one f32->f16 cast of the s_o row.
  - ScalarE: ONLY the 16 sigmoid ACTIVATEs (~2 us each, (N+352)/1.2GHz),
    bias port adds s_s[t*128+p] per row-block; sigmoid ACT table preloaded
    by a dummy activation at t=0 so the ~1.3us table load is off-path.
  - PE: rank-1 b seed + s_o broadcast into PSUM [128, 2048].
  - Geometric chunks on input (early DVE start) and output (early first
    store, short last-store tail).
"""

import numpy as np

import concourse.mybir as mybir
from concourse import bacc, bass_utils
from concourse.tile import TileContext

P = 128          # partitions
D = 512          # feature dim
S = 2048         # sents
NT = S // P      # 16 tiles per half
OCHUNKS = [1, 1, 2, 4, 4, 2, 1, 1]   # orig tiles per chunk (small tail ->
                                     # short scatter/cast chain to psum_so)
SCH = 4          # simp tiles per chunk
NSC = NT // SCH
OGROUPS = [1, 1, 2, 4, 4, 2, 1, 1]   # output row-tiles per store
NCORES = 8
F32 = mybir.dt.float32
F16 = mybir.dt.float16


def _kernel_body(tc, out, x, w, bvec):
    nc = tc.nc
    # orig half, partition-outer: i = p*NT + n (contiguous input lines)
    xo_re = x[0:S, :].rearrange("(p n) d -> p n d", n=NT)
    # simp half, partition-inner: j = n*P + p  (bias needs column layout)
    xs_re = x[S:2 * S, :].rearrange("(n p) d -> p n d", p=P)

    with (
        tc.tile_pool(name="consts", bufs=1) as cpool,
        tc.tile_pool(name="xin", bufs=1) as xpool,
        tc.tile_pool(name="scratch", bufs=4) as spool,
        tc.tile_pool(name="outsm", bufs=2) as ospool,
        tc.tile_pool(name="outbig", bufs=2) as obpool,
        tc.tile_pool(name="psum", bufs=1, space="PSUM") as ppool,
    ):
        # preload the sigmoid ACT table while DMAs run (dummy activation)
        dummy = cpool.tile([1, 1], F32, tag="dummy")
        nc.gpsimd.memset(dummy, 0.0)
        nc.scalar.activation(dummy, dummy,
                             mybir.ActivationFunctionType.Sigmoid)

        # --- orig input stream: geometric chunks, all in flight at once ---
        xo_tiles = []
        n0 = 0
        for c, sz in enumerate(OCHUNKS):
            xo = xpool.tile([P, sz, D], F16, tag=f"xo{c}", name=f"xo{c}")
            nc.sync.dma_start(out=xo, in_=xo_re[:, n0:n0 + sz, :])
            xo_tiles.append(xo)
            n0 += sz

        # simp tiles; their loads go on the Sync queue behind the orig
        # chunks but are gated by a tiny DVE write into each tile (WAW dep)
        # so the transfers only start once phase 1a is nearly done -- an
        # ungated DMA would be scheduled at t=0 and starve the orig stream.
        xs_tiles = [
            xpool.tile([P, SCH, D], F16, tag=f"xs{g}", name=f"xs{g}")
            for g in range(NSC)
        ]

        # w / b replicated across partitions by zero-stride DMA (SWDGE);
        # w_o first since it gates the first multiply.
        w_bc = cpool.tile([P, 2 * D], F16, tag="wbc")
        nc.gpsimd.dma_start(out=w_bc[:, 0:D],
                            in_=w[:, 0:D].broadcast_to([P, D]))
        nc.gpsimd.dma_start(out=w_bc[:, D:2 * D],
                            in_=w[:, D:2 * D].broadcast_to([P, D]))
        ones_row = cpool.tile([1, P], F16, tag="ones")
        nc.gpsimd.memset(ones_row, 1.0)

        s_o_mat = cpool.tile([P, NT], F32, tag="somat")   # s_o[p*16+n] @ [p,n]
        s_sb_mat = cpool.tile([P, NT], F32, tag="ssmat")  # s_s, col t
        so_rowf = cpool.tile([1, S], F32, tag="sorowf")   # f32 scatter dest
        so_row = cpool.tile([1, S], F16, tag="sorow")
        b_sb = cpool.tile([1, 1], F32, tag="bsb")
        nc.sync.dma_start(out=b_sb, in_=bvec)
        b_row = cpool.tile([1, 512], F16, tag="brow")
        nc.gpsimd.memset(b_row, 0.0)
        nc.vector.tensor_scalar_add(b_row, b_row, b_sb)
        sob_psum = ppool.tile([P, S], F32, tag="sob")     # s_o + b, every row

        # PSUM b seed can run as soon as b_row is ready (start=True)
        for j in range(S // 512):
            nc.tensor.matmul(sob_psum[:, j * 512:(j + 1) * 512], ones_row,
                             b_row, start=True, stop=False)

        # --- phase 1a: orig half -> s_o (DVE mul+reduce) -> so_row ---
        n0 = 0
        for c, sz in enumerate(OCHUNKS):
            xo = xo_tiles[c]
            for blk in range(sz):
                t = n0 + blk
                prod = spool.tile([P, D], F16, tag="prod", name=f"po{t}")
                nc.vector.tensor_mul(out=prod, in0=xo[:, blk, :],
                                     in1=w_bc[:, 0:D])
                nc.vector.tensor_reduce(
                    s_o_mat[:, t:t + 1], prod,
                    axis=mybir.AxisListType.X, op=mybir.AluOpType.add)
                if t in (8, 10, 12, 14):
                    g = {8: 0, 10: 1, 12: 2, 14: 3}[t]
                    nc.vector.tensor_copy(
                        out=xs_tiles[g][0:1, 0, 0:1],
                        in_=prod[0:1, 0:1])
            src = s_o_mat[:, n0:n0 + sz]
            dst = so_rowf.rearrange("o (p n) -> o p n", n=NT)[:, :, n0:n0 + sz]
            nc.scalar.dma_start(out=dst, in_=src)
            n0 += sz

        # f32 -> fp16 cast on DVE (keeps ScalarE free for sigmoids)
        nc.vector.tensor_copy(out=so_row, in_=so_rowf)

        # simp loads: queued on Sync behind the orig chunks, released by the
        # gate writes above
        for g in range(NSC):
            nc.sync.dma_start(out=xs_tiles[g],
                              in_=xs_re[:, g * SCH:(g + 1) * SCH, :])

        # --- s_o broadcast across partitions via rank-1 matmuls on top of
        # the b seed ---
        for j in range(S // 512):
            nc.tensor.matmul(sob_psum[:, j * 512:(j + 1) * 512], ones_row,
                             so_row[:, j * 512:(j + 1) * 512],
                             start=False, stop=True)

        # --- phase 1b + 2: simp half -> s_s, then sigmoid row-blocks ---
        # output store groups (geometric both ends)
        group_of_tile = []
        for gi, gsz in enumerate(OGROUPS):
            group_of_tile += [gi] * gsz
        group_start = np.cumsum([0] + OGROUPS).tolist()

        o_sb = None
        for g in range(NSC):
            xs = xs_tiles[g]
            for blk in range(SCH):
                t = g * SCH + blk
                prod = spool.tile([P, D], F16, tag="prod", name=f"ps{t}")
                nc.vector.tensor_mul(out=prod, in0=xs[:, blk, :],
                                     in1=w_bc[:, D:2 * D])
                nc.vector.tensor_reduce(
                    s_sb_mat[:, t:t + 1], prod,
                    axis=mybir.AxisListType.X, op=mybir.AluOpType.add)
            for blk in range(SCH):
                t = g * SCH + blk
                gi = group_of_tile[t]
                gsz = OGROUPS[gi]
                q = t - group_start[gi]
                if q == 0:
                    pool = ospool if gsz <= 2 else obpool
                    o_sb = pool.tile([P, gsz, S], F16, tag=f"osb{gsz}",
                                     name=f"og{gi}")
                nc.scalar.activation(
                    o_sb[:, q, :], sob_psum,
                    mybir.ActivationFunctionType.Sigmoid,
                    bias=s_sb_mat[:, t:t + 1],
                    scale=1.0,
                )
                if q == gsz - 1:
                    r0 = group_start[gi] * P
                    if gsz == 1:
                        nc.sync.dma_start(out=out[r0:r0 + P, :],
                                          in_=o_sb[:, 0, :])
                    else:
                        dst = out[r0:r0 + gsz * P, :].rearrange(
                            "(q p) i -> p q i", p=P)
                        nc.sync.dma_start(out=dst, in_=o_sb)


def build_program():
    nc = bacc.Bacc(
        "TRN2",
        debug=False,
        target_bir_lowering=False,
        num_devices=NCORES,
    )
    x = nc.dram_tensor("x", [2 * S, D], F16, kind="ExternalInput").ap()
    w = nc.dram_tensor("w", [1, 2 * D], F16, kind="ExternalInput").ap()
    bvec = nc.dram_tensor("bvec", [1, 1], F32, kind="ExternalInput").ap()
    out = nc.dram_tensor("out", [S, S], F16, kind="ExternalOutput").ap()
    with TileContext(nc) as tc:
        _kernel_body(tc, out, x, w, bvec)
    nc.compile()
    return nc


_PROGRAM = None


def _get_program():
    global _PROGRAM
    if _PROGRAM is None:
        _PROGRAM = build_program()
    return _PROGRAM


def make_in_maps(prop_state, W, b):
    prop = np.asarray(prop_state, dtype=np.float32).astype(np.float16)
    prop = np.ascontiguousarray(prop)
    w = np.ascontiguousarray(
        np.asarray(W, dtype=np.float32).reshape(1, 2 * D).astype(np.float16))
    bv = np.ascontiguousarray(np.asarray(b, dtype=np.float32).reshape(1, 1))
    assert prop.shape == (NCORES, 2 * S, D), prop.shape
    return [{"x": prop[i], "w": w, "bvec": bv} for i in range(NCORES)]


def kernel(A, prop_state, W, b, _trace=False):
    nc = _get_program()
    in_maps = make_in_maps(prop_state, W, b)
    res = bass_utils.run_bass_kernel_spmd(
        nc, in_maps, core_ids=list(range(NCORES)), trace=_trace)
    out = np.stack([res.results[i]["out"] for i in range(NCORES)], axis=0)
    if _trace:
        kernel.last_results = res
    return out.astype(np.float32)
